# revision 53
# baseline (speedup 1.0000x reference)
"""LocallyConnected2dV2 Trainium2 kernel (bf16, raw pre-context input stream).

Math: out[b, a, bp] = sum_{k,l} xpad[b, 5a+k, 5bp+l] * kw[a, bp, k, l] + bias[a, bp]

Strategy (8 cores, data-parallel over batch, 128 images/core):
  - Host: transpose each core's x shard to [col j', row r, batch b]; cast
    bf16; compact W into per-image-row banded blocks Wh[r, j', 50] (bf16).
  - Input stream: ALL combined x+w chunk DMAs plus the aux (bias/ones)
    DMA are raw-bass instructions issued BEFORE the TileContext: they
    bypass the context-entry prologue and drain strictly FIFO on the two
    HWDGE rings (sync: even chunks + aux; scalar: odd chunks).
  - PE schedule is ROW-MAJOR with overlapped psum groups: psum groups
    g0..g3 cover SIX output rows a (150 cols) and g4a/g4b cover 4/2, so
    every image row r's {a0, a0+1} window lands wholly inside one group
    -> exactly ONE Ldweights+Matmult (50 cols, zero-padded at edges) per
    image row, 128 total.  Boundary output rows a=5,10,15,20,23 are
    computed twice (once per adjacent group, bias only in the lower);
    the host adds the two copies when unsharding.
  - Chunk gating: a const-operand dummy matmul carrying a raw semaphore
    wait precedes each chunk's first consumer (in-order dispatch blocks
    the next matmul's Ldweights, which would otherwise read the chunk
    before that matmul's own wait).  Every consuming matmul ALSO carries
    the chunk wait (attached post-scheduling - the Tile sim cannot model
    the external DMAs): measured, this keeps the issue order aligned
    with arrival order and is worth ~2.5us.
  - Outputs in 3 pieces: O1 (g0..g2) + O2 (g3+g4a) on the sync ring,
    whose FIFO orders them after ALL input chunks (an output transfer
    overlapping the tail chunks would delay their sems and stall PE);
    O3 (g4b, 50 cols) is cast on the Act engine and DMA'd from the same
    engine, so the end-of-kernel chain (stop->cast->gen->transfer) has
    no cross-engine hop and minimal gen time.
"""

import numpy as np
import ml_dtypes

BF16 = ml_dtypes.bfloat16

B = 1024
R = 128           # image rows = cols
NCORES = 8
BS = B // NCORES  # 128 batch per core
NK = 625
WP = 132
# chunk row spans: medium head (PE start ~10.5us, no engine starvation),
# big middle, shrinking tail (each receipt overlaps the next transfer)
CHUNK_LEN = [10, 10, 16, 16, 16, 16, 16, 16, 4, 4, 2, 2]
# chunks alternate rings; outputs sit FIFO-behind each ring's inputs so
# the engines only reach output descriptors after all input chunks — an
# output transfer overlapping the tail chunks would delay their sems
SYNC_CHUNKS = set(range(len(CHUNK_LEN)))
CHUNK_OFF = [0]
for _n in CHUNK_LEN[:-1]:
    CHUNK_OFF.append(CHUNK_OFF[-1] + _n)
NCH = len(CHUNK_LEN)
assert sum(CHUNK_LEN) == R

# psum groups: (first a, n_a, bias lo slice, zero-pad cols) — groups overlap
# by one output row a; every row r's {a0,a0+1} window fits one group.
#   g0: a0..5 (150 cols), g1: a5..10, g2: a10..15, g3: a15..20,
#   g4a: a20..23 (100), g4b: a23..24 (50)
GROUPS = [
    # (a_base, ncols, bias_src_lo, bias_src_hi, bias_dst_off)
    (0, 150, 0, 150, 0),      # bias a0..5
    (5, 150, 150, 275, 25),   # bias a6..10 (a5 copy unbiased)
    (10, 150, 275, 400, 25),
    (15, 150, 400, 525, 25),
    (20, 100, 525, 600, 25),  # g4a: a20..23, bias a21..23
    (23, 50, 600, 625, 25),   # g4b: a23..24, bias a24
]
NG = len(GROUPS)
OUT_OFF = [0]
for (_, w, _, _, _) in GROUPS:
    OUT_OFF.append(OUT_OFF[-1] + w)
OUT_W = OUT_OFF[-1]           # 750
AUX_BIAS_W = sum(w for (_, w, _, _, _) in GROUPS)   # 750
AUX_W = AUX_BIAS_W + BS       # + ones vector


def _chunk_of_row(r):
    for ic in range(NCH):
        if CHUNK_OFF[ic] <= r < CHUNK_OFF[ic] + CHUNK_LEN[ic]:
            return ic
    raise ValueError(r)


def _a0_of_row(r):
    return min(max((r - 3) // 5, 0), 23)


def _group_of_row(r):
    """Group index and psum col offset for row r's 50-wide window."""
    a0 = _a0_of_row(r)
    if a0 >= 23:
        g = 5
    elif a0 >= 20:
        g = 4
    else:
        g = a0 // 5
    return g, (a0 - GROUPS[g][0]) * 25


def _last_row_of_group(g):
    return max(r for r in range(R) if _group_of_row(r)[0] == g)


def prep_weights(W, bias):
    """W [17424, 625], bias [25,25] -> wt [128, 128*50], aux [1, AUX_W]."""
    W = np.asarray(W, np.float32)
    i = np.arange(NK)
    si = (i // 25) * 5
    sj = (i % 25) * 5
    rows = ((si[:, None, None] + np.arange(10)[None, :, None]) * WP
            + sj[:, None, None] + np.arange(10)[None, None, :])
    kw = W[rows.reshape(NK, 100), i[:, None]].reshape(25, 25, 10, 10)

    r = np.arange(R)[:, None, None]
    jp = np.arange(R)[None, :, None]
    c = np.arange(50)[None, None, :]
    ai = c // 25
    bp = c % 25
    a = np.clip((r - 3) // 5, 0, 23) + ai
    k = r + 2 - 5 * a
    l = jp + 2 - 5 * bp
    valid = (k >= 0) & (k < 10) & (l >= 0) & (l < 10)
    Wh = np.where(valid, kw[a, bp, np.clip(k, 0, 9), np.clip(l, 0, 9)], 0.0)
    Wh = Wh.astype(np.float32)                       # [r, j', 50]
    wt = np.ascontiguousarray(Wh.transpose(1, 0, 2)).reshape(R, R * 50)

    bias_f = np.asarray(bias, np.float32).reshape(NK)
    parts = []
    for (_, w, blo, bhi, boff) in GROUPS:
        slot = np.zeros(w, np.float32)
        slot[boff:boff + (bhi - blo)] = bias_f[blo:bhi]
        parts.append(slot)
    bias_slots = np.concatenate(parts)               # [AUX_BIAS_W]
    return wt, bias_slots


def _build_nc():
    import concourse.bass as bass
    import concourse.mybir as mybir
    import concourse.tile as tile
    from concourse import bacc

    bf16 = mybir.dt.bfloat16
    nc = bacc.Bacc("TRN2", target_bir_lowering=False, debug=False)
    xw_cols = sum(n * (BS + 50) for n in CHUNK_LEN)
    xw = nc.dram_tensor("xw", [R, xw_cols], bf16, kind="ExternalInput").ap()
    aux = nc.dram_tensor("aux", [1, AUX_W], bf16, kind="ExternalInput").ap()
    # O1 = g0..g2 (450 cols); O2 = g3+g4a (250); O3 = g4b (50, the short
    # final chain).  Host concatenates the three pieces.
    out1 = nc.dram_tensor("out1", [BS, OUT_OFF[3]], bf16,
                          kind="ExternalOutput").ap()
    out2 = nc.dram_tensor("out2", [1, BS, 1, OUT_OFF[5] - OUT_OFF[3]], bf16,
                          kind="ExternalOutput").ap()
    out3 = nc.dram_tensor("out3", [1, BS, 1, OUT_OFF[6] - OUT_OFF[5]], bf16,
                          kind="ExternalOutput").ap()

    # raw pre-context input stream
    es = [nc.alloc_sbuf_tensor(f"e{i}", [R, CHUNK_LEN[i] * (BS + 50)], bf16)
          for i in range(NCH)]
    aux_sb = nc.alloc_sbuf_tensor("auxsb", [1, AUX_W], bf16)
    o2_sb = nc.alloc_sbuf_tensor(
        "o2sb", [BS, 1, 1, OUT_OFF[5] - OUT_OFF[3]], bf16)
    o3_sb = nc.alloc_sbuf_tensor(
        "o3sb", [BS, 1, 1, OUT_OFF[6] - OUT_OFF[5]], bf16)
    sems = [nc.alloc_semaphore(f"esem{i}") for i in range(NCH)]
    saux = nc.alloc_semaphore("sauxsem")
    xw_off = [0]
    for n in CHUNK_LEN:
        xw_off.append(xw_off[-1] + n * (BS + 50))

    def chunk_dma(i):
        eng = nc.sync if i in SYNC_CHUNKS else nc.scalar
        eng.dma_start(es[i].ap(),
                      xw[:, xw_off[i]:xw_off[i + 1]]).then_inc(sems[i], 16)
    chunk_dma(0)
    chunk_dma(1)
    nc.sync.dma_start(aux_sb.ap(), aux[:]).then_inc(saux, 16)
    for i in range(2, NCH):
        chunk_dma(i)

    ones_t = aux_sb.ap()[:, AUX_BIAS_W:AUX_W]
    o2_ap = o2_sb.ap()
    o3_ap = o3_sb.ap()

    waits = []   # (inst, sem) — raw waits attached post-scheduling

    with tile.TileContext(nc) as tc:
        with (
            tc.tile_pool(name="ps", bufs=1, space=bass.MemorySpace.PSUM) as ps_pool,
            tc.tile_pool(name="dps", bufs=1, space=bass.MemorySpace.PSUM) as dps_pool,
            tc.tile_pool(name="ob", bufs=1) as ob_pool,
        ):
            one_bf = nc.const_aps.aps[(mybir.dt.bfloat16, 1.0)]
            dps = dps_pool.tile([1, 1], mybir.dt.float32, tag="dummy")
            out_sb = ob_pool.tile([BS, OUT_OFF[3]], bf16, tag="osb")
            # full-bank psum tiles so each group owns a bank
            ps = []
            for g in range(NG):
                pst = ps_pool.tile([BS, 512], mybir.dt.float32,
                                   tag=f"ps{g}", name=f"ps{g}")
                ps.append(pst)

            def gate(sem):
                # Const-operand dummy matmul carrying the raw wait: blocks
                # in-order dispatch so the NEXT matmul's Ldweights (which
                # executes before that matmul's own wait) cannot read the
                # chunk early.
                inst = nc.tensor.matmul(
                    dps[0:1, 0:1], one_bf[0:1, 0:1], one_bf[0:1, 0:1],
                    start=True, stop=True, skip_group_check=True)
                waits.append((inst, sem))

            # bias first (start matmuls) — aux rides the sync ring early
            gate(saux)
            for g, (_, w, _, _, _) in enumerate(GROUPS):
                inst = nc.tensor.matmul(
                    ps[g][:, 0:w], ones_t,
                    aux_sb.ap()[:, OUT_OFF[g]:OUT_OFF[g] + w],
                    start=True, stop=False)
                waits.append((inst, saux))

            last_rows = [_last_row_of_group(g) for g in range(NG)]
            seen = set()
            for r in range(R):
                ic = _chunk_of_row(r)
                if ic not in seen:
                    gate(sems[ic])
                    seen.add(ic)
                rr = r - CHUNK_OFF[ic]
                ct = es[ic].ap()
                lhsT = ct[:, rr * BS:(rr + 1) * BS]
                wb = CHUNK_LEN[ic] * BS + rr * 50
                g, pc = _group_of_row(r)
                inst = nc.tensor.matmul(
                    ps[g][:, pc:pc + 50], lhsT, ct[:, wb:wb + 50],
                    start=False, stop=(r == last_rows[g]))
                waits.append((inst, sems[ic]))
                if r == last_rows[g]:
                    w = GROUPS[g][1]
                    if g <= 2:
                        nc.vector.tensor_copy(
                            out_sb[:, OUT_OFF[g]:OUT_OFF[g] + w],
                            ps[g][:, 0:w])
                    elif g <= 4:
                        o = OUT_OFF[g] - OUT_OFF[3]
                        nc.vector.tensor_copy(
                            o2_ap[:, 0, 0, o:o + w], ps[g][:, 0:w])
                    else:
                        # final cast on the Act engine: same sequencer as
                        # the O3 DMA, so no cross-engine hop on the tail
                        nc.scalar.copy(
                            o3_ap[:, 0, 0, 0:w], ps[g][:, 0:w])
                    if g == 2:
                        # g0..g2 in one DMA on the sync ring: sync carries
                        # the final input chunk, so out1's descriptors sit
                        # FIFO-behind ALL inputs and never steal engines
                        # from the in-flight tail whose sems gate PE.
                        nc.sync.dma_start(
                            out1[:, 0:OUT_OFF[3]], out_sb[:, 0:OUT_OFF[3]])
                    elif g == 4:
                        nc.sync.dma_start(
                            out2[0, :, 0, :], o2_ap[:, 0, 0, :])
                    elif g == 5:
                        nc.scalar.dma_start(
                            out3[0, :, 0, :], o3_ap[:, 0, 0, :])



    # Attach the stream waits post-scheduling: the Tile simulator does not
    # model the pre-context DMAs and would report a false deadlock.
    for inst, sem in waits:
        inst._wait_ge(sem, 16)
    nc.compile()
    return nc


_NC_CACHE = []


def _get_nc():
    if not _NC_CACHE:
        _NC_CACHE.append(_build_nc())
    return _NC_CACHE[0]


def make_in_maps(x, W, bias):
    x = np.asarray(x, np.float32)
    wt, bias_slots = prep_weights(W, bias)
    wt16 = wt.astype(BF16)
    auxv = np.concatenate(
        [bias_slots.astype(BF16), np.ones(BS, BF16)]).reshape(1, AUX_W)
    in_maps = []
    for c in range(NCORES):
        xc = x[c * BS:(c + 1) * BS]                      # [b, r, j']
        xtv = np.ascontiguousarray(
            xc.transpose(2, 1, 0)).astype(BF16).reshape(R, R * BS)
        parts = []
        for ic in range(NCH):
            o, n = CHUNK_OFF[ic], CHUNK_LEN[ic]
            parts.append(xtv[:, o * BS:(o + n) * BS])
            parts.append(wt16[:, o * 50:(o + n) * 50])
        xwv = np.ascontiguousarray(np.concatenate(parts, axis=1))
        in_maps.append({"xw": xwv, "aux": auxv})
    return in_maps


def _assemble(o):
    """o [BS, OUT_W] float32 -> [BS, 25, 25] with seam adds."""
    res = np.zeros((o.shape[0], 25, 25), np.float32)
    for g, (a_base, w, _, _, _) in enumerate(GROUPS):
        blk = o[:, OUT_OFF[g]:OUT_OFF[g] + w].reshape(o.shape[0], w // 25, 25)
        for j in range(w // 25):
            res[:, a_base + j, :] += blk[:, j, :]
    return res


def run(x, W, bias, trace=False, **kw):
    from concourse import bass_utils
    nc = _get_nc()
    res = bass_utils.run_bass_kernel_spmd(
        nc, make_in_maps(x, W, bias), list(range(NCORES)), trace=trace, **kw)
    outs = []
    for c in range(NCORES):
        o = np.concatenate(
            [np.asarray(res.results[c]["out1"]).reshape(BS, -1),
             np.asarray(res.results[c]["out2"]).reshape(BS, -1),
             np.asarray(res.results[c]["out3"]).reshape(BS, -1)],
            axis=1).astype(np.float32)
        outs.append(_assemble(o))
    return np.concatenate(outs, axis=0), res


def kernel(**inputs):
    out, _ = run(inputs["x"], inputs["W"], inputs["bias"])
    return out


# revision 55
# speedup vs baseline: 1.0536x; 1.0536x over previous
"""LocallyConnected2dV2 Trainium2 kernel (bf16, raw pre-context input stream).

Math: out[b, a, bp] = sum_{k,l} xpad[b, 5a+k, 5bp+l] * kw[a, bp, k, l] + bias[a, bp]

Strategy (8 cores, data-parallel over batch, 128 images/core):
  - Host: transpose each core's x shard to [col j', row r, batch b]; cast
    bf16; compact W into per-image-row banded blocks Wh[r, j', 50] (bf16).
  - Input stream: ALL combined x+w chunk DMAs plus the aux (bias/ones)
    DMA are raw-bass instructions issued BEFORE the TileContext: they
    bypass the context-entry prologue and drain strictly FIFO on the two
    HWDGE rings (sync: even chunks + aux; scalar: odd chunks).
  - PE schedule is ROW-MAJOR with overlapped psum groups: psum groups
    g0..g3 cover SIX output rows a (150 cols) and g4a/g4b cover 4/2, so
    every image row r's {a0, a0+1} window lands wholly inside one group
    -> exactly ONE Ldweights+Matmult (50 cols, zero-padded at edges) per
    image row, 128 total.  Boundary output rows a=5,10,15,20,23 are
    computed twice (once per adjacent group, bias only in the lower);
    the host adds the two copies when unsharding.
  - Chunk gating: a const-operand dummy matmul carrying a raw semaphore
    wait precedes each chunk's first consumer (in-order dispatch blocks
    the next matmul's Ldweights, which would otherwise read the chunk
    before that matmul's own wait applies).  Every consuming matmul ALSO
    carries the chunk wait (attached post-scheduling - the Tile sim
    cannot model the external DMAs); dropping them measured ~2.5us
    slower (the Tile scheduler then de-aligns issue order from arrival).
  - Outputs in 3 pieces, each ring-FIFO-ordered behind that ring's input
    chunks so output descriptors never steal engines from in-flight
    input (an output transfer overlapping the tail chunks delays their
    sems and stalls PE): O1 (g0..g2, scalar ring), O2 (g3+g4a, sync
    ring), O3 (g4b, 50 cols) cast on the Act engine and DMA'd from that
    same engine so the final stop->cast->gen->transfer chain has no
    cross-engine hop.
"""

import numpy as np
import ml_dtypes

BF16 = ml_dtypes.bfloat16

B = 1024
R = 128           # image rows = cols
NCORES = 8
BS = B // NCORES  # 128 batch per core
NK = 625
WP = 132
# chunk row spans: medium head (PE start ~10.5us, no engine starvation),
# big middle, shrinking tail (each receipt overlaps the next transfer)
CHUNK_LEN = [8, 14, 16, 16, 16, 16, 16, 8, 6, 4, 4, 2, 2]
# chunks alternate rings; outputs sit FIFO-behind each ring's inputs so
# the engines only reach output descriptors after all input chunks — an
# output transfer overlapping the tail chunks would delay their sems
SYNC_CHUNKS = {i for i in range(len(CHUNK_LEN)) if i % 2 == 0}
CHUNK_OFF = [0]
for _n in CHUNK_LEN[:-1]:
    CHUNK_OFF.append(CHUNK_OFF[-1] + _n)
NCH = len(CHUNK_LEN)
assert sum(CHUNK_LEN) == R

# psum groups: (first a, n_a, bias lo slice, zero-pad cols) — groups overlap
# by one output row a; every row r's {a0,a0+1} window fits one group.
#   g0: a0..5 (150 cols), g1: a5..10, g2: a10..15, g3: a15..20,
#   g4a: a20..23 (100), g4b: a23..24 (50)
GROUPS = [
    # (a_base, ncols, bias_src_lo, bias_src_hi, bias_dst_off)
    (0, 150, 0, 150, 0),      # bias a0..5
    (5, 150, 150, 275, 25),   # bias a6..10 (a5 copy unbiased)
    (10, 150, 275, 400, 25),
    (15, 150, 400, 525, 25),
    (20, 100, 525, 600, 25),  # g4a: a20..23, bias a21..23
    (23, 50, 600, 625, 25),   # g4b: a23..24, bias a24
]
NG = len(GROUPS)
OUT_OFF = [0]
for (_, w, _, _, _) in GROUPS:
    OUT_OFF.append(OUT_OFF[-1] + w)
OUT_W = OUT_OFF[-1]           # 750
AUX_BIAS_W = sum(w for (_, w, _, _, _) in GROUPS)   # 750
AUX_W = AUX_BIAS_W + BS       # + ones vector


def _chunk_of_row(r):
    for ic in range(NCH):
        if CHUNK_OFF[ic] <= r < CHUNK_OFF[ic] + CHUNK_LEN[ic]:
            return ic
    raise ValueError(r)


def _a0_of_row(r):
    return min(max((r - 3) // 5, 0), 23)


def _group_of_row(r):
    """Group index and psum col offset for row r's 50-wide window."""
    a0 = _a0_of_row(r)
    if a0 >= 23:
        g = 5
    elif a0 >= 20:
        g = 4
    else:
        g = a0 // 5
    return g, (a0 - GROUPS[g][0]) * 25


def _last_row_of_group(g):
    return max(r for r in range(R) if _group_of_row(r)[0] == g)


def prep_weights(W, bias):
    """W [17424, 625], bias [25,25] -> wt [128, 128*50], aux [1, AUX_W]."""
    W = np.asarray(W, np.float32)
    i = np.arange(NK)
    si = (i // 25) * 5
    sj = (i % 25) * 5
    rows = ((si[:, None, None] + np.arange(10)[None, :, None]) * WP
            + sj[:, None, None] + np.arange(10)[None, None, :])
    kw = W[rows.reshape(NK, 100), i[:, None]].reshape(25, 25, 10, 10)

    r = np.arange(R)[:, None, None]
    jp = np.arange(R)[None, :, None]
    c = np.arange(50)[None, None, :]
    ai = c // 25
    bp = c % 25
    a = np.clip((r - 3) // 5, 0, 23) + ai
    k = r + 2 - 5 * a
    l = jp + 2 - 5 * bp
    valid = (k >= 0) & (k < 10) & (l >= 0) & (l < 10)
    Wh = np.where(valid, kw[a, bp, np.clip(k, 0, 9), np.clip(l, 0, 9)], 0.0)
    Wh = Wh.astype(np.float32)                       # [r, j', 50]
    wt = np.ascontiguousarray(Wh.transpose(1, 0, 2)).reshape(R, R * 50)

    bias_f = np.asarray(bias, np.float32).reshape(NK)
    parts = []
    for (_, w, blo, bhi, boff) in GROUPS:
        slot = np.zeros(w, np.float32)
        slot[boff:boff + (bhi - blo)] = bias_f[blo:bhi]
        parts.append(slot)
    bias_slots = np.concatenate(parts)               # [AUX_BIAS_W]
    return wt, bias_slots


def _build_nc():
    import concourse.bass as bass
    import concourse.mybir as mybir
    import concourse.tile as tile
    from concourse import bacc

    bf16 = mybir.dt.bfloat16
    nc = bacc.Bacc("TRN2", target_bir_lowering=False, debug=False)
    xw_cols = sum(n * (BS + 50) for n in CHUNK_LEN)
    xw = nc.dram_tensor("xw", [R, xw_cols], bf16, kind="ExternalInput").ap()
    aux = nc.dram_tensor("aux", [1, AUX_W], bf16, kind="ExternalInput").ap()
    # O1 = g0..g2 (450 cols, plain HWDGE); O2 = g3+g4a (250); O3 = g4b (50).
    # out2/out3 are 4D [batch, dhi, dho, n_ctx] so kv_writeback's stride
    # asserts hold; host concatenates the three pieces.
    out1 = nc.dram_tensor("out1", [BS, OUT_OFF[3]], bf16,
                          kind="ExternalOutput").ap()
    out2 = nc.dram_tensor("out2", [1, BS, 1, OUT_OFF[5] - OUT_OFF[3]], bf16,
                          kind="ExternalOutput").ap()
    out3 = nc.dram_tensor("out3", [1, BS, 1, OUT_OFF[6] - OUT_OFF[5]], bf16,
                          kind="ExternalOutput").ap()

    # raw pre-context input stream
    es = [nc.alloc_sbuf_tensor(f"e{i}", [R, CHUNK_LEN[i] * (BS + 50)], bf16)
          for i in range(NCH)]
    aux_sb = nc.alloc_sbuf_tensor("auxsb", [1, AUX_W], bf16)
    # SWDGE-written output staging (4D so the in_ap stride checks hold)
    o2_sb = nc.alloc_sbuf_tensor(
        "o2sb", [BS, 1, 1, OUT_OFF[5] - OUT_OFF[3]], bf16)
    o3_sb = nc.alloc_sbuf_tensor(
        "o3sb", [BS, 1, 1, OUT_OFF[6] - OUT_OFF[5]], bf16)
    idx_sb = nc.alloc_sbuf_tensor("idx0", [BS, 1], mybir.dt.int32)
    gp_scr = nc.alloc_sbuf_tensor("gpscr", [1, 2], bf16)
    sems = [nc.alloc_semaphore(f"esem{i}") for i in range(NCH)]
    saux = nc.alloc_semaphore("sauxsem")
    prep_sem = nc.alloc_semaphore("prepsem")
    o2_sem = nc.alloc_semaphore("o2sem")    # O2 transfer complete
    o3_sem = nc.alloc_semaphore("o3sem")    # O3 transfer complete
    xw_off = [0]
    for n in CHUNK_LEN:
        xw_off.append(xw_off[-1] + n * (BS + 50))

    def chunk_dma(i):
        eng = nc.sync if i in SYNC_CHUNKS else nc.scalar
        eng.dma_start(es[i].ap(),
                      xw[:, xw_off[i]:xw_off[i + 1]]).then_inc(sems[i], 16)
    chunk_dma(0)
    chunk_dma(1)
    nc.sync.dma_start(aux_sb.ap(), aux[:]).then_inc(saux, 16)
    for i in range(2, NCH):
        chunk_dma(i)

    ones_t = aux_sb.ap()[:, AUX_BIAS_W:AUX_W]
    o2_ap = o2_sb.ap()
    o3_ap = o3_sb.ap()
    idx_ap = idx_sb.ap()
    gp_scr_ap = gp_scr.ap()
    o2w = OUT_OFF[5] - OUT_OFF[3]
    o3w = OUT_OFF[6] - OUT_OFF[5]

    waits = []   # (inst, sem) — raw waits attached post-scheduling

    with tile.TileContext(nc) as tc:
        with (
            tc.tile_pool(name="ps", bufs=1, space=bass.MemorySpace.PSUM) as ps_pool,
            tc.tile_pool(name="dps", bufs=1, space=bass.MemorySpace.PSUM) as dps_pool,
            tc.tile_pool(name="ob", bufs=1) as ob_pool,
        ):
            one_bf = nc.const_aps.aps[(mybir.dt.bfloat16, 1.0)]
            dps = dps_pool.tile([1, 1], mybir.dt.float32, tag="dummy")
            out_sb = ob_pool.tile([BS, OUT_OFF[3]], bf16, tag="osb")
            # full-bank psum tiles so each group owns a bank
            ps = []
            for g in range(NG):
                pst = ps_pool.tile([BS, 512], mybir.dt.float32,
                                   tag=f"ps{g}", name=f"ps{g}")
                ps.append(pst)

            def gate(sem):
                # Const-operand dummy matmul carrying the raw wait: blocks
                # in-order dispatch so the NEXT matmul's Ldweights (which
                # executes before that matmul's own wait) cannot read the
                # chunk early.
                inst = nc.tensor.matmul(
                    dps[0:1, 0:1], one_bf[0:1, 0:1], one_bf[0:1, 0:1],
                    start=True, stop=True, skip_group_check=True)
                waits.append((inst, sem))

            # bias first (start matmuls) — aux rides the sync ring early
            gate(saux)
            for g, (_, w, _, _, _) in enumerate(GROUPS):
                inst = nc.tensor.matmul(
                    ps[g][:, 0:w], ones_t,
                    aux_sb.ap()[:, OUT_OFF[g]:OUT_OFF[g] + w],
                    start=True, stop=False)
                waits.append((inst, saux))

            last_rows = [_last_row_of_group(g) for g in range(NG)]
            seen = set()
            for r in range(R):
                ic = _chunk_of_row(r)
                if ic not in seen:
                    gate(sems[ic])
                    seen.add(ic)
                rr = r - CHUNK_OFF[ic]
                ct = es[ic].ap()
                lhsT = ct[:, rr * BS:(rr + 1) * BS]
                wb = CHUNK_LEN[ic] * BS + rr * 50
                g, pc = _group_of_row(r)
                inst = nc.tensor.matmul(
                    ps[g][:, pc:pc + 50], lhsT, ct[:, wb:wb + 50],
                    start=False, stop=(r == last_rows[g]))
                waits.append((inst, sems[ic]))
                if r == last_rows[g]:
                    w = GROUPS[g][1]
                    if g <= 2:
                        nc.vector.tensor_copy(
                            out_sb[:, OUT_OFF[g]:OUT_OFF[g] + w],
                            ps[g][:, 0:w])
                    elif g <= 4:
                        o = OUT_OFF[g] - OUT_OFF[3]
                        nc.vector.tensor_copy(
                            o2_ap[:, 0, 0, o:o + w], ps[g][:, 0:w])
                    else:
                        # final cast on the Act engine: same sequencer as
                        # the O3 DMA, so no cross-engine hop on the tail
                        nc.scalar.copy(
                            o3_ap[:, 0, 0, 0:w], ps[g][:, 0:w])
                    if g == 2:
                        # g0..g2 in one DMA on the scalar ring: its FIFO
                        # naturally orders it after scalar's input chunks,
                        # so it never steals engines from in-flight input.
                        nc.scalar.dma_start(
                            out1[:, 0:OUT_OFF[3]], out_sb[:, 0:OUT_OFF[3]])
                    elif g == 4:
                        nc.sync.dma_start(
                            out2[0, :, 0, :], o2_ap[:, 0, 0, :])
                    elif g == 5:
                        nc.scalar.dma_start(
                            out3[0, :, 0, :], o3_ap[:, 0, 0, :])



    # Attach the stream waits post-scheduling: the Tile simulator does not
    # model the pre-context DMAs and would report a false deadlock.
    for inst, sem in waits:
        inst._wait_ge(sem, 16)
    nc.compile()
    return nc


_NC_CACHE = []


def _get_nc():
    if not _NC_CACHE:
        _NC_CACHE.append(_build_nc())
    return _NC_CACHE[0]


def make_in_maps(x, W, bias):
    x = np.asarray(x, np.float32)
    wt, bias_slots = prep_weights(W, bias)
    wt16 = wt.astype(BF16)
    auxv = np.concatenate(
        [bias_slots.astype(BF16), np.ones(BS, BF16)]).reshape(1, AUX_W)
    in_maps = []
    for c in range(NCORES):
        xc = x[c * BS:(c + 1) * BS]                      # [b, r, j']
        xtv = np.ascontiguousarray(
            xc.transpose(2, 1, 0)).astype(BF16).reshape(R, R * BS)
        parts = []
        for ic in range(NCH):
            o, n = CHUNK_OFF[ic], CHUNK_LEN[ic]
            parts.append(xtv[:, o * BS:(o + n) * BS])
            parts.append(wt16[:, o * 50:(o + n) * 50])
        xwv = np.ascontiguousarray(np.concatenate(parts, axis=1))
        in_maps.append({"xw": xwv, "aux": auxv})
    return in_maps


def _assemble(o):
    """o [BS, OUT_W] float32 -> [BS, 25, 25] with seam adds."""
    res = np.zeros((o.shape[0], 25, 25), np.float32)
    for g, (a_base, w, _, _, _) in enumerate(GROUPS):
        blk = o[:, OUT_OFF[g]:OUT_OFF[g] + w].reshape(o.shape[0], w // 25, 25)
        for j in range(w // 25):
            res[:, a_base + j, :] += blk[:, j, :]
    return res


def run(x, W, bias, trace=False, **kw):
    from concourse import bass_utils
    nc = _get_nc()
    res = bass_utils.run_bass_kernel_spmd(
        nc, make_in_maps(x, W, bias), list(range(NCORES)), trace=trace, **kw)
    outs = []
    for c in range(NCORES):
        o = np.concatenate(
            [np.asarray(res.results[c]["out1"]).reshape(BS, -1),
             np.asarray(res.results[c]["out2"]).reshape(BS, -1),
             np.asarray(res.results[c]["out3"]).reshape(BS, -1)],
            axis=1).astype(np.float32)
        outs.append(_assemble(o))
    return np.concatenate(outs, axis=0), res


def kernel(**inputs):
    out, _ = run(inputs["x"], inputs["W"], inputs["bias"])
    return out
